# revision 39
# baseline (speedup 1.0000x reference)
"""Multi-head attention (S=2048, B=2, D=1024, H=16) on 8 Trainium2 NeuronCores.

Sharding: batch x heads. Core c handles batch c//4 and heads (c%4)*4..+4,
processed as two head-pairs that map onto a pipelined attention loop
(scores row-tiled per head pair, softmax denominator via a ones-column in V,
QKV projections restricted to the core's 256 output dims, row-parallel
output projection accumulated over both pairs in PSUM). The host sums the
4 partial outputs per batch and adds bo.

On-device compute is fp16 with fp32 PSUM accumulation; output partials are
written fp16 (fp8 anywhere in the matmul chain fails the 2e-2 tolerance —
measured 2.4e-2..8.7e-2 per placement). x loads stream column-chunk-major,
deadline-ordered across the 3 DMA-issuing engines (big loads on the HWDGE
sync/scalar queues; the slower SWDGE gpsimd queue carries only smalls +
xv_c0/c1 + wo0; xv_c2/c3 are issued late on scalar so they cannot be
hoisted in front of the ACT evacuations). The PE p-state warmup runs on a
memset junk tile immediately at t=0 with no DMA dependency, sized to end
when xk chunk 0 lands. Q/K projection evacuations ride the ACT engine
(Identity shares the exp activation table, bias as a per-partition AP);
V-projection evacuation is 2 fused DVE tensor_tensors per token tile.
Unit 0 is DMA-bound end-to-end (~12MB of input gates it), so it also
absorbs the deadline-free pair-1 K projections in its stall bubbles.
"""

import math

import numpy as np

S, B, D, H = 2048, 2, 1024, 16
DK = D // H               # 64
NCORES = 8
HLOC = 4                  # heads per core
NP = 2                    # head pairs per core
DLOC = HLOC * DK          # local output dims per core = 256
KT = D // 128             # contraction tiles = 8
NQC = S // 512            # query chunks = 4
NKB = S // 128            # key blocks = 16
NTT = S // 128            # token tiles = 16
SCALE = 1.0 / math.sqrt(DK)

_prog_cache = {}


def _build(masked: bool):
    import concourse.mybir as mybir
    import concourse.tile as tile
    from concourse import bacc

    f16 = mybir.dt.float16
    f32 = mybir.dt.float32
    EXP = mybir.ActivationFunctionType.Exp
    MUL = mybir.AluOpType.mult
    ADD = mybir.AluOpType.add

    nc = bacc.Bacc("TRN2", target_bir_lowering=False, debug=False)

    def din(name, shape, dt=f16):
        return nc.dram_tensor(name, shape, dt, kind="ExternalInput").ap()

    xq = din("xq", [D, S])             # query^T, this core's batch
    xk = din("xk", [D, S])
    xv = din("xv", [D, S])
    # projection weights prearranged: w_arr[p, kt, m] = W[hs+m, kt*128+p]
    wq = din("wq", [128, KT * DLOC])
    wk = din("wk", [128, KT * DLOC])
    wv = din("wv", [128, KT * DLOC])
    wo = din("wo", [DLOC, D])          # Wo[:, hs:hs+256].T
    bq = din("bq", [DLOC], f32)
    bk = din("bk", [DLOC], f32)
    bv = din("bv", [DLOC], f32)
    mb = din("mb", [S], f32)           # additive mask bias per key (0 / -1e30)
    out = nc.dram_tensor("out", [S, D], f16, kind="ExternalOutput").ap()

    with tile.TileContext(nc) as tc:
        with (
            tc.tile_pool(name="wsb", bufs=1) as wsb,
            tc.tile_pool(name="xsb", bufs=1) as xsb,
            tc.tile_pool(name="qkv", bufs=1) as qkv,
            tc.tile_pool(name="esb", bufs=6) as esb,
            tc.tile_pool(name="nrm", bufs=3) as nrm,
            tc.tile_pool(name="osb", bufs=4) as osb,
            tc.tile_pool(name="pj", bufs=2, space="PSUM") as pj,
            tc.tile_pool(name="psc", bufs=2, space="PSUM") as psc,
            tc.tile_pool(name="pcx", bufs=1, space="PSUM") as pcx,
        ):
            # ---- warmup junk tile: no DMA dependency, memset at t=0 ------
            junk = wsb.tile([128, 640], f16, tag="junk")
            nc.vector.memset(junk[:, 0:320], 0.0)
            nc.gpsimd.memset(junk[:, 320:640], 0.0)

            # ---- weight / bias / mask DMAs + x loads, column-chunk-major -
            # only sync (HWDGE), scalar (HWDGE), gpsimd (SWDGE) can issue
            # DMA, and each queue serializes issue+completion, so order each
            # queue by deadline: wk/wq/wv first, then x column chunks.
            xk_b = xsb.tile([128, KT, S], f16, tag="xk", name="xk_b")
            xq_b = xsb.tile([128, KT, S], f16, tag="xq", name="xq_b")
            xv_b = xsb.tile([128, KT, S], f16, tag="xv", name="xv_b")
            xk_t = [xk_b[:, kt, :] for kt in range(KT)]
            xq_t = [xq_b[:, kt, :] for kt in range(KT)]
            xv_t = [xv_b[:, kt, :] for kt in range(KT)]
            xkr = xk.rearrange("(kt p) s -> p kt s", p=128)
            xqr = xq.rearrange("(kt p) s -> p kt s", p=128)
            xvr = xv.rearrange("(kt p) s -> p kt s", p=128)

            # Queue split: the Scalar engine must be free for the K/Q
            # evacuations + exp as soon as chunk-0 data lands, so it issues
            # only wq/xq_c0/xk_c1 (a blocked 3rd issue still unblocks by
            # ~9us). Sync (no compute duties) carries the deep xk/xq tail;
            # gpsimd (SWDGE, pipelined issue) carries xv + small loads.
            def cs(c):
                return slice(c * 512, (c + 1) * 512)

            # sync (HWDGE, no compute duties) leads with xk chunk 0 so the
            # first K projection can start ~14us in; wk/wq ride scalar ahead
            # of xq_c0 (wk is only needed as the stationary operand). gpsimd
            # (slower SWDGE) carries the small loads + xv_c0/c1 + wo0.
            nc.sync.dma_start(out=xk_b[:, :, cs(0)], in_=xkr[:, :, cs(0)])
            wk_sb = wsb.tile([128, KT, DLOC], f16, tag="wk")
            nc.scalar.dma_start(out=wk_sb, in_=wk.rearrange("p (kt m) -> p kt m", kt=KT))
            wq_sb = wsb.tile([128, KT, DLOC], f16, tag="wq")
            nc.scalar.dma_start(out=wq_sb, in_=wq.rearrange("p (kt m) -> p kt m", kt=KT))
            wv_sb = wsb.tile([128, KT, DLOC], f16, tag="wv")
            nc.gpsimd.dma_start(out=wv_sb, in_=wv.rearrange("p (kt m) -> p kt m", kt=KT))
            w_sb = {"wq": wq_sb, "wk": wk_sb, "wv": wv_sb}

            nc.scalar.dma_start(out=xq_b[:, :, cs(0)], in_=xqr[:, :, cs(0)])
            # small bias/mask loads first on gpsimd (Q/K evac needs bq/bk)
            bq_sb = wsb.tile([128, NP], f32, tag="bq")
            nc.gpsimd.dma_start(out=bq_sb, in_=bq.rearrange("(pr i) -> i pr", pr=NP))
            bk_sb = wsb.tile([128, NP], f32, tag="bk")
            nc.gpsimd.dma_start(out=bk_sb, in_=bk.rearrange("(pr i) -> i pr", pr=NP))
            bv_row = wsb.tile([1, DLOC], f32, tag="bv_row")
            nc.gpsimd.dma_start(out=bv_row, in_=bv.unsqueeze(0))
            mb_sb = wsb.tile([128, NKB], f32, tag="mb")
            nc.gpsimd.dma_start(out=mb_sb, in_=mb.rearrange("(kb p) -> p kb", p=128))
            bv_bc = wsb.tile([128, DLOC], f32, tag="bv_bc")
            nc.gpsimd.partition_broadcast(bv_bc, bv_row)
            nc.gpsimd.dma_start(out=xv_b[:, :, cs(0)], in_=xvr[:, :, cs(0)])
            # deep tails, deadline-ordered; xv_c2/c3 are issued late on
            # scalar (emitted after the Q(0,0) evac, so they never block
            # the ACT ops)
            nc.sync.dma_start(out=xk_b[:, :, cs(1)], in_=xkr[:, :, cs(1)])
            nc.scalar.dma_start(out=xq_b[:, :, cs(1)], in_=xqr[:, :, cs(1)])
            nc.gpsimd.dma_start(out=xv_b[:, :, cs(1)], in_=xvr[:, :, cs(1)])
            nc.sync.dma_start(out=xk_b[:, :, cs(2)], in_=xkr[:, :, cs(2)])
            nc.sync.dma_start(out=xk_b[:, :, cs(3)], in_=xkr[:, :, cs(3)])
            nc.sync.dma_start(out=xq_b[:, :, cs(2)], in_=xqr[:, :, cs(2)])
            nc.sync.dma_start(out=xq_b[:, :, cs(3)], in_=xqr[:, :, cs(3)])
            wo_sb = [wsb.tile([128, D], f16, tag=f"wo{p}", name=f"wo{p}")
                     for p in range(NP)]
            nc.gpsimd.dma_start(out=wo_sb[0], in_=wo[0:128, :])
            nc.sync.dma_start(out=wo_sb[1], in_=wo[128:256, :])

            # ---- persistent per-pair activations -------------------------
            qT = [qkv.tile([128, S], f16, tag=f"qT{p}", name=f"qT{p}") for p in range(NP)]
            kT = [qkv.tile([128, S], f16, tag=f"kT{p}", name=f"kT{p}") for p in range(NP)]
            vv = [qkv.tile([128, 2, NKB, 68], f16, tag=f"vv{p}", name=f"vv{p}")
                  for p in range(NP)]
            for p in range(NP):
                nc.vector.memset(vv[p][:, :, :, 64:65], 1.0)
            ctxn = [qkv.tile([128, S], f16, tag=f"ctxn{p}", name=f"ctxn{p}")
                    for p in range(NP)]

            # ---- projections ---------------------------------------------
            # QK evacuation rides the ACT engine (Identity shares the exp
            # table, per-partition bias AP); V evacuation is 2 fused DVE
            # tensor_tensors per token tile. Pool cannot touch PSUM.
            IDENT = mybir.ActivationFunctionType.Identity

            def proj_qk_chunk(p, which, qc, dve_evac=False):
                """Single (pair, chunk) Q/K projection through the pj pool.
                Evacuation on ACT by default; dve_evac=True moves it to the
                DVE for projections injected into ACT-floor units."""
                w, bias, dst, xt = (("wq", bq_sb, qT, xq_t) if which == "q"
                                    else ("wk", bk_sb, kT, xk_t))
                ps = pj.tile([128, 512], f32, tag="pj", name="ps")
                sl = cs(qc)
                for kt in range(KT):
                    nc.tensor.matmul(ps, w_sb[w][:, kt, p * 128:(p + 1) * 128],
                                     xt[kt][:, sl],
                                     start=(kt == 0), stop=(kt == KT - 1))
                if dve_evac:
                    nc.vector.tensor_scalar(out=dst[p][:, sl], in0=ps,
                                            scalar1=bias[:, p:p + 1],
                                            scalar2=None, op0=ADD)
                else:
                    nc.scalar.activation(dst[p][:, sl], ps, IDENT,
                                         bias=bias[:, p:p + 1])

            def proj_v_tt(tts):
                """V projection for token tiles tts, both pairs at once."""
                for tt in tts:
                    ps = pj.tile([128, 512], f32, tag="pj", name="ps")
                    sl = slice(tt * 128, (tt + 1) * 128)
                    for kt in range(KT):
                        nc.tensor.matmul(ps[:, 0:DLOC], xv_t[kt][:, sl],
                                         w_sb["wv"][:, kt, :],
                                         start=(kt == 0), stop=(kt == KT - 1))
                    for p in range(NP):
                        nc.vector.tensor_tensor(
                            out=vv[p][:, :, tt, 0:64],
                            in0=ps[:, p * 128:(p + 1) * 128],
                            in1=bv_bc[:, p * 128:(p + 1) * 128], op=ADD)

            def outproj_tt(tts):
                for tt in tts:
                    tsl = slice(tt * 128, (tt + 1) * 128)
                    for eh in range(2):
                        po = pj.tile([128, 512], f32, tag="pj", name="po")
                        esl = slice(eh * 512, (eh + 1) * 512)
                        for p in range(NP):
                            nc.tensor.matmul(po, ctxn[p][:, tsl], wo_sb[p][:, esl],
                                             start=(p == 0), stop=(p == NP - 1))
                        oc = osb.tile([128, 512], f16, tag="oc", name="oc")
                        nc.vector.tensor_copy(oc, po)
                        nc.sync.dma_start(out=out[tsl, esl], in_=oc)

            # ---- attention ----------------------------------------------
            # kb-granular pipeline: one psco tile [128, h0-q|h1-q] per key
            # block, ring of 2 => scores(kb) only waits on exp(kb-2), which
            # the ACT engine finished long ago. One exp ACTIVATE per kb
            # covers both heads (and the mask bias, when present).
            def attn_qc(p, qc, injects=(), last=False, pj_head=False):
                injects = list(injects)
                qsl = cs(qc)
                pctx = [pcx.tile([65, 512], f32, tag=f"cx{h}", name=f"cx{h}")
                        for h in range(2)]

                def scores_kb(kb, use_pj=False):
                    ksl = slice(kb * 128, (kb + 1) * 128)
                    if use_pj:
                        # kb0/kb1 through the pj pool: the unit's start no
                        # longer waits on the previous unit's exp(14)/(15)
                        # freeing the psc ring
                        ts = [pj.tile([128, 512], f32, tag="pj", name="scp")
                              for _ in range(2)]
                        for h in range(2):
                            hsl = slice(h * 64, (h + 1) * 64)
                            nc.tensor.matmul(
                                ts[h], kT[p][hsl, ksl], qT[p][hsl, qsl],
                                start=True, stop=True,
                                tile_position=(h * 64, 0))
                        return tuple(ts)
                    psco = psc.tile([128, 1024], f32, tag="sc", name="sc")
                    for h in range(2):
                        hsl = slice(h * 64, (h + 1) * 64)
                        nc.tensor.matmul(
                            psco[:, h * 512:(h + 1) * 512],
                            kT[p][hsl, ksl], qT[p][hsl, qsl],
                            start=True, stop=True,
                            tile_position=(h * 64, 0))
                    return psco

                def exp_ctx_kb(kb, psco):
                    et = esb.tile([128, 1024], f16, tag="e", name="et")
                    if isinstance(psco, tuple):
                        for h in range(2):
                            esl = slice(h * 512, (h + 1) * 512)
                            if masked:
                                nc.scalar.activation(et[:, esl], psco[h], EXP,
                                                     bias=mb_sb[:, kb:kb + 1],
                                                     scale=SCALE)
                            else:
                                nc.scalar.activation(et[:, esl], psco[h], EXP,
                                                     scale=SCALE)
                    elif masked:
                        nc.scalar.activation(et, psco, EXP,
                                             bias=mb_sb[:, kb:kb + 1], scale=SCALE)
                    else:
                        nc.scalar.activation(et, psco, EXP, scale=SCALE)
                    for h in range(2):
                        nc.tensor.matmul(
                            pctx[h], vv[p][:, h, kb, 0:65],
                            et[:, h * 512:(h + 1) * 512],
                            start=(kb == 0), stop=(kb == NKB - 1))

                ring = [scores_kb(0, use_pj=pj_head),
                        scores_kb(1, use_pj=pj_head)]
                for kb in range(2, NKB):
                    ring.append(scores_kb(kb))
                    if injects:
                        f = injects.pop(0)
                        if f is not None:
                            f()
                    exp_ctx_kb(kb - 2, ring.pop(0))
                while injects:
                    f = injects.pop(0)
                    if f is not None:
                        f()
                exp_ctx_kb(NKB - 2, ring.pop(0))
                exp_ctx_kb(NKB - 1, ring.pop(0))

                # evacuate pctx first (both heads) so the next unit's ctx
                # chain gets its PSUM banks back ~2.5us earlier; the
                # recip/broadcast/multiply then run off the SBUF copies.
                # In the LAST unit nothing reuses the banks, so skip the
                # cd copies (multiply straight from PSUM) and put the cl
                # copies on the now-idle scalar engine.
                cds, cls = [], []
                for h in range(2):
                    if last:
                        # nothing reuses the banks: multiply straight from
                        # PSUM; only the denominator row is copied out (the
                        # DVE reciprocal can't read PSUM)
                        cds.append(pctx[h][0:64, :])
                        cl = nrm.tile([1, 512], f32, tag=f"cl{h}", name="cl")
                        nc.scalar.copy(cl, pctx[h][64:65, :])
                        cls.append(cl)
                    else:
                        cd = nrm.tile([64, 512], f32, tag=f"cd{h}", name="cd")
                        nc.vector.tensor_copy(cd, pctx[h][0:64, :])
                        cds.append(cd)
                        cl = nrm.tile([1, 512], f32, tag=f"cl{h}", name="cl")
                        nc.vector.tensor_copy(cl, pctx[h][64:65, :])
                        cls.append(cl)
                for h in range(2):
                    hsl = slice(h * 64, (h + 1) * 64)
                    rl = nrm.tile([1, 512], f32, tag="rl", name="rl")
                    nc.vector.reciprocal_approx_fast(rl, cls[h])
                    rl_bc = nrm.tile([64, 512], f32, tag="rlb", name="rlb")
                    nc.gpsimd.partition_broadcast(rl_bc, rl)
                    nc.vector.tensor_tensor(out=ctxn[p][hsl, qsl],
                                            in0=cds[h], in1=rl_bc, op=MUL)

            # ---- prologue: immediate PE p-state warmup on the junk tile,
            # then chunk-0 K/Q projections as soon as their DMAs land.
            # enough junk matmuls to hold the PE p-state at full clock until
            # xk chunk 0 lands (~21us); each costs ~215ns hot, and at most
            # one junk matmul of latency is added once real work is ready.
            ksc = psc.tile([128, 1024], f32, tag="sc", name="ksc")
            for wu in range(42):
                nc.tensor.matmul(ksc[:, 0:512], junk[:, 0:128],
                                 junk[:, 128:640],
                                 start=True, stop=True)
            proj_qk_chunk(0, "k", 0)
            proj_qk_chunk(0, "q", 0)
            # late-issued xv_c2/c3 on scalar: emitted after the Q(0,0)
            # evac so the scheduler is less likely to hoist them in front
            # of the ACT evacuations
            nc.scalar.dma_start(out=xv_b[:, :, cs(2)], in_=xvr[:, :, cs(2)])
            nc.scalar.dma_start(out=xv_b[:, :, cs(3)], in_=xvr[:, :, cs(3)])

            # ---- attention schedule with injected projection work -------
            # unit 0 carries the V projection (deadline: exp_ctx(kb) needs
            # vv[kb]) plus just-in-time K chunks (scores(kb) needs chunk
            # kb//4) and Q(0,1); later units are balanced ~6.6us each.
            # emission-order deadlines inside unit 0: inject[i] is emitted
            # after scores(i+2), so K(c1) needs i<=1 (scores(4) reads kT
            # chunk 1), K(c2) i<=5, K(c3) i<=9, V(t,t+1) i<=t (exp_ctx(t)
            # reads vv[t]).
            attn_qc(0, 0, [
                lambda: proj_v_tt([0, 1]),
                lambda: proj_qk_chunk(0, "k", 1),
                lambda: proj_v_tt([2, 3]),
                lambda: proj_v_tt([4, 5]),
                lambda: proj_qk_chunk(0, "k", 2),
                lambda: proj_v_tt([6, 7]),
                lambda: proj_v_tt([8, 9]),
                lambda: proj_v_tt([10, 11]),
                lambda: proj_qk_chunk(0, "k", 3),
                lambda: proj_v_tt([12, 13]),
                lambda: proj_v_tt([14, 15]),
                lambda: proj_qk_chunk(0, "q", 1),
                # pair-1 K projections ride unit0's DMA-stall bubbles
                lambda: proj_qk_chunk(1, "k", 0),
                lambda: proj_qk_chunk(1, "k", 1),
            ])
            attn_qc(0, 1, [
                lambda: proj_qk_chunk(0, "q", 2, dve_evac=True),
                lambda: proj_qk_chunk(0, "q", 3, dve_evac=True),
            ])
            attn_qc(0, 2, [
                lambda: proj_qk_chunk(1, "k", 2, dve_evac=True),
                lambda: proj_qk_chunk(1, "k", 3, dve_evac=True),
            ])
            attn_qc(0, 3, [
                lambda: proj_qk_chunk(1, "q", 0, dve_evac=True),
                lambda: proj_qk_chunk(1, "q", 1, dve_evac=True),
            ])
            attn_qc(1, 0, [
                lambda: proj_qk_chunk(1, "q", 2, dve_evac=True),
                lambda: proj_qk_chunk(1, "q", 3, dve_evac=True),
            ])
            # outproj injected a few slots in so it never stalls the PE on
            # the previous unit's normalize chain (ctxn written ~3us after
            # that unit's last exp)
            attn_qc(1, 1, [
                None,
                None,
                lambda: outproj_tt([0]),
                None,
                lambda: outproj_tt([1]),
                None,
                lambda: outproj_tt([2]),
                None,
                lambda: outproj_tt([3]),
            ])
            attn_qc(1, 2, [
                None,
                None,
                lambda: outproj_tt([4]),
                None,
                lambda: outproj_tt([5]),
                None,
                lambda: outproj_tt([6]),
                None,
                lambda: outproj_tt([7]),
            ])
            attn_qc(1, 3, [
                None,
                None,
                lambda: outproj_tt([8]),
                None,
                lambda: outproj_tt([9]),
                None,
                lambda: outproj_tt([10]),
                None,
                lambda: outproj_tt([11]),
            ], last=True)

            def outproj_tail(tts, store_engs):
                for tt in tts:
                    tsl = slice(tt * 128, (tt + 1) * 128)
                    po = psc.tile([128, 1024], f32, tag="sc", name="po2")
                    for eh in range(2):
                        esl = slice(eh * 512, (eh + 1) * 512)
                        for p in range(NP):
                            nc.tensor.matmul(po[:, esl], ctxn[p][:, tsl],
                                             wo_sb[p][:, esl],
                                             start=(p == 0), stop=(p == NP - 1))
                    for eh in range(2):
                        esl = slice(eh * 512, (eh + 1) * 512)
                        oc = osb.tile([128, 512], f16, tag="oc", name="oc")
                        nc.vector.tensor_copy(oc, po[:, esl])
                        store_engs[eh].dma_start(out=out[tsl, esl], in_=oc)

            outproj_tail([12, 13], store_engs=[nc.scalar, nc.gpsimd])
            outproj_tail([14, 15], store_engs=[nc.sync, nc.gpsimd])

    nc.compile()
    return nc


def _get_prog(masked: bool):
    key = masked
    if key not in _prog_cache:
        _prog_cache[key] = _build(masked)
    return _prog_cache[key]


def make_in_maps(query, key, value, mask, Wq, bq, Wk, bk, Wv, bv, Wo, bo):
    query = np.asarray(query)
    key = np.asarray(key)
    value = np.asarray(value)
    mask = np.asarray(mask)
    Wq, bq = np.asarray(Wq), np.asarray(bq)
    Wk, bk = np.asarray(Wk), np.asarray(bk)
    Wv, bv = np.asarray(Wv), np.asarray(bv)
    Wo = np.asarray(Wo)

    def t16(x):  # [S, B, D] -> contiguous [D, B, S] fp16
        return np.ascontiguousarray(x.transpose(2, 1, 0).astype(np.float16))

    def warr(W, hs):  # [128, KT*DLOC]: row p = concat_kt W[hs+m, kt*128+p]
        wt = W[hs:hs + DLOC, :].T.astype(np.float16)       # [kt*128+p, m]
        return np.ascontiguousarray(
            wt.reshape(KT, 128, DLOC).transpose(1, 0, 2).reshape(128, KT * DLOC))

    xq3, xk3, xv3 = t16(query), t16(key), t16(value)
    xqb = [np.ascontiguousarray(xq3[:, b, :]) for b in range(B)]
    xkb = [np.ascontiguousarray(xk3[:, b, :]) for b in range(B)]
    xvb = [np.ascontiguousarray(xv3[:, b, :]) for b in range(B)]
    mbias = np.where(mask.reshape(S), 0.0, -1e30).astype(np.float32)

    wqs = [warr(Wq, g * DLOC) for g in range(4)]
    wks = [warr(Wk, g * DLOC) for g in range(4)]
    wvs = [warr(Wv, g * DLOC) for g in range(4)]
    wos = [np.ascontiguousarray(Wo[:, g * DLOC:(g + 1) * DLOC].T.astype(np.float16))
           for g in range(4)]

    in_maps = []
    for c in range(NCORES):
        b, g = c // 4, c % 4
        hs = g * DLOC
        in_maps.append({
            "xq": xqb[b], "xk": xkb[b], "xv": xvb[b],
            "wq": wqs[g], "wk": wks[g], "wv": wvs[g], "wo": wos[g],
            "bq": bq[hs:hs + DLOC].astype(np.float32),
            "bk": bk[hs:hs + DLOC].astype(np.float32),
            "bv": bv[hs:hs + DLOC].astype(np.float32),
            "mb": mbias,
        })
    return in_maps


def kernel(query, key, value, mask, Wq, bq, Wk, bk, Wv, bv, Wo, bo):
    from concourse.bass_utils import run_bass_kernel_spmd

    mask = np.asarray(mask)
    bo = np.asarray(bo)
    masked = not bool(mask.all())
    nc = _get_prog(masked)
    in_maps = make_in_maps(query, key, value, mask, Wq, bq, Wk, bk, Wv, bv, Wo, bo)

    res = run_bass_kernel_spmd(nc, in_maps, core_ids=list(range(NCORES)))
    acc = np.zeros((S, B, D), dtype=np.float64)
    for c in range(NCORES):
        acc[:, c // 4, :] += res.results[c]["out"].astype(np.float64)
    acc += bo.astype(np.float64)
    return acc.astype(np.float32)


# revision 40
# speedup vs baseline: 1.1722x; 1.1722x over previous
"""Multi-head attention (S=2048, B=2, D=1024, H=16) on 8 Trainium2 NeuronCores.

Sharding: batch x heads. Core c handles batch c//4 and heads (c%4)*4..+4,
processed as two head-pairs that map onto a pipelined attention loop
(scores row-tiled per head pair, softmax denominator via a ones-column in V,
QKV projections restricted to the core's 256 output dims, row-parallel
output projection accumulated over both pairs in PSUM). The host sums the
4 partial outputs per batch and adds bo.

On-device compute is fp16 with fp32 PSUM accumulation; output partials are
written fp16 (fp8 anywhere in the matmul chain fails the 2e-2 tolerance —
measured 2.4e-2..8.7e-2 per placement). x loads stream column-chunk-major,
deadline-ordered across the 3 DMA-issuing engines (big loads on the HWDGE
sync/scalar queues; the slower SWDGE gpsimd queue carries only smalls +
xv_c0/c1 + wo0; xv_c2/c3 are issued late on scalar so they cannot be
hoisted in front of the ACT evacuations). The PE p-state warmup runs on a
memset junk tile immediately at t=0 with no DMA dependency, sized to end
when xk chunk 0 lands. Q/K projection evacuations ride the ACT engine
(Identity shares the exp activation table, bias as a per-partition AP);
V-projection evacuation is 2 fused DVE tensor_tensors per token tile.
Unit 0 is DMA-bound end-to-end (~12MB of input gates it), so it also
absorbs the deadline-free pair-1 K projections in its stall bubbles.
"""

import math

import numpy as np

S, B, D, H = 2048, 2, 1024, 16
DK = D // H               # 64
NCORES = 8
HLOC = 4                  # heads per core
NP = 2                    # head pairs per core
DLOC = HLOC * DK          # local output dims per core = 256
KT = D // 128             # contraction tiles = 8
NQC = S // 512            # query chunks = 4
NKB = S // 128            # key blocks = 16
NTT = S // 128            # token tiles = 16
SCALE = 1.0 / math.sqrt(DK)

_prog_cache = {}


def _build(masked: bool):
    import concourse.mybir as mybir
    import concourse.tile as tile
    from concourse import bacc

    f16 = mybir.dt.float16
    f32 = mybir.dt.float32
    EXP = mybir.ActivationFunctionType.Exp
    MUL = mybir.AluOpType.mult
    ADD = mybir.AluOpType.add

    nc = bacc.Bacc("TRN2", target_bir_lowering=False, debug=False)

    def din(name, shape, dt=f16):
        return nc.dram_tensor(name, shape, dt, kind="ExternalInput").ap()

    xq = din("xq", [D, S])             # query^T, this core's batch
    xk = din("xk", [D, S])
    xv = din("xv", [D, S])
    # projection weights prearranged: w_arr[p, kt, m] = W[hs+m, kt*128+p]
    wq = din("wq", [128, KT * DLOC])
    wk = din("wk", [128, KT * DLOC])
    wv = din("wv", [128, KT * DLOC])
    wo = din("wo", [DLOC, D])          # Wo[:, hs:hs+256].T
    bq = din("bq", [DLOC], f32)
    bk = din("bk", [DLOC], f32)
    bv = din("bv", [DLOC], f32)
    mb = din("mb", [S], f32)           # additive mask bias per key (0 / -1e30)
    out = nc.dram_tensor("out", [S, D], f16, kind="ExternalOutput").ap()

    with tile.TileContext(nc) as tc:
        with (
            tc.tile_pool(name="wsb", bufs=1) as wsb,
            tc.tile_pool(name="xsb", bufs=1) as xsb,
            tc.tile_pool(name="qkv", bufs=1) as qkv,
            tc.tile_pool(name="esb", bufs=6) as esb,
            tc.tile_pool(name="nrm", bufs=3) as nrm,
            tc.tile_pool(name="osb", bufs=4) as osb,
            tc.tile_pool(name="pj", bufs=2, space="PSUM") as pj,
            tc.tile_pool(name="psc", bufs=2, space="PSUM") as psc,
            tc.tile_pool(name="pcx", bufs=1, space="PSUM") as pcx,
        ):
            # ---- warmup junk tile: no DMA dependency, memset at t=0 ------
            junk = wsb.tile([128, 640], f16, tag="junk")
            nc.vector.memset(junk[:, 0:320], 0.0)
            nc.gpsimd.memset(junk[:, 320:640], 0.0)

            # ---- weight / bias / mask DMAs + x loads, column-chunk-major -
            # only sync (HWDGE), scalar (HWDGE), gpsimd (SWDGE) can issue
            # DMA, and each queue serializes issue+completion, so order each
            # queue by deadline: wk/wq/wv first, then x column chunks.
            xk_b = xsb.tile([128, KT, S], f16, tag="xk", name="xk_b")
            xq_b = xsb.tile([128, KT, S], f16, tag="xq", name="xq_b")
            xv_b = xsb.tile([128, KT, S], f16, tag="xv", name="xv_b")
            xk_t = [xk_b[:, kt, :] for kt in range(KT)]
            xq_t = [xq_b[:, kt, :] for kt in range(KT)]
            xv_t = [xv_b[:, kt, :] for kt in range(KT)]
            xkr = xk.rearrange("(kt p) s -> p kt s", p=128)
            xqr = xq.rearrange("(kt p) s -> p kt s", p=128)
            xvr = xv.rearrange("(kt p) s -> p kt s", p=128)

            # Queue split: the Scalar engine must be free for the K/Q
            # evacuations + exp as soon as chunk-0 data lands, so it issues
            # only wq/xq_c0/xk_c1 (a blocked 3rd issue still unblocks by
            # ~9us). Sync (no compute duties) carries the deep xk/xq tail;
            # gpsimd (SWDGE, pipelined issue) carries xv + small loads.
            def cs(c):
                return slice(c * 512, (c + 1) * 512)

            # sync (HWDGE, no compute duties) leads with xk chunk 0 so the
            # first K projection can start ~14us in; wk/wq ride scalar ahead
            # of xq_c0 (wk is only needed as the stationary operand). gpsimd
            # (slower SWDGE) carries the small loads + xv_c0/c1 + wo0.
            nc.sync.dma_start(out=xk_b[:, :, cs(0)], in_=xkr[:, :, cs(0)])
            wk_sb = wsb.tile([128, KT, DLOC], f16, tag="wk")
            nc.scalar.dma_start(out=wk_sb, in_=wk.rearrange("p (kt m) -> p kt m", kt=KT))
            wq_sb = wsb.tile([128, KT, DLOC], f16, tag="wq")
            nc.scalar.dma_start(out=wq_sb, in_=wq.rearrange("p (kt m) -> p kt m", kt=KT))
            wv_sb = wsb.tile([128, KT, DLOC], f16, tag="wv")
            nc.gpsimd.dma_start(out=wv_sb, in_=wv.rearrange("p (kt m) -> p kt m", kt=KT))
            w_sb = {"wq": wq_sb, "wk": wk_sb, "wv": wv_sb}

            nc.scalar.dma_start(out=xq_b[:, :, cs(0)], in_=xqr[:, :, cs(0)])
            # small bias/mask loads first on gpsimd (Q/K evac needs bq/bk)
            bq_sb = wsb.tile([128, NP], f32, tag="bq")
            nc.gpsimd.dma_start(out=bq_sb, in_=bq.rearrange("(pr i) -> i pr", pr=NP))
            bk_sb = wsb.tile([128, NP], f32, tag="bk")
            nc.gpsimd.dma_start(out=bk_sb, in_=bk.rearrange("(pr i) -> i pr", pr=NP))
            bv_row = wsb.tile([1, DLOC], f32, tag="bv_row")
            nc.gpsimd.dma_start(out=bv_row, in_=bv.unsqueeze(0))
            mb_sb = wsb.tile([128, NKB], f32, tag="mb")
            nc.gpsimd.dma_start(out=mb_sb, in_=mb.rearrange("(kb p) -> p kb", p=128))
            bv_bc = wsb.tile([128, DLOC], f32, tag="bv_bc")
            nc.gpsimd.partition_broadcast(bv_bc, bv_row)
            nc.gpsimd.dma_start(out=xv_b[:, :, cs(0)], in_=xvr[:, :, cs(0)])
            # deep tails, deadline-ordered; xv_c2/c3 are issued late on
            # scalar (emitted after the Q(0,0) evac, so they never block
            # the ACT ops)
            nc.sync.dma_start(out=xk_b[:, :, cs(1)], in_=xkr[:, :, cs(1)])
            nc.scalar.dma_start(out=xq_b[:, :, cs(1)], in_=xqr[:, :, cs(1)])
            nc.gpsimd.dma_start(out=xv_b[:, :, cs(1)], in_=xvr[:, :, cs(1)])
            nc.sync.dma_start(out=xk_b[:, :, cs(2)], in_=xkr[:, :, cs(2)])
            nc.sync.dma_start(out=xk_b[:, :, cs(3)], in_=xkr[:, :, cs(3)])
            nc.sync.dma_start(out=xq_b[:, :, cs(2)], in_=xqr[:, :, cs(2)])
            nc.sync.dma_start(out=xq_b[:, :, cs(3)], in_=xqr[:, :, cs(3)])
            wo_sb = [wsb.tile([128, D], f16, tag=f"wo{p}", name=f"wo{p}")
                     for p in range(NP)]
            nc.gpsimd.dma_start(out=wo_sb[0], in_=wo[0:128, :])
            nc.sync.dma_start(out=wo_sb[1], in_=wo[128:256, :])

            # ---- persistent per-pair activations -------------------------
            qT = [qkv.tile([128, S], f16, tag=f"qT{p}", name=f"qT{p}") for p in range(NP)]
            kT = [qkv.tile([128, S], f16, tag=f"kT{p}", name=f"kT{p}") for p in range(NP)]
            vv = [qkv.tile([128, 2, NKB, 68], f16, tag=f"vv{p}", name=f"vv{p}")
                  for p in range(NP)]
            for p in range(NP):
                nc.vector.memset(vv[p][:, :, :, 64:65], 1.0)
            ctxn = [qkv.tile([128, S], f16, tag=f"ctxn{p}", name=f"ctxn{p}")
                    for p in range(NP)]

            # ---- projections ---------------------------------------------
            # QK evacuation rides the ACT engine (Identity shares the exp
            # table, per-partition bias AP); V evacuation is 2 fused DVE
            # tensor_tensors per token tile. Pool cannot touch PSUM.
            IDENT = mybir.ActivationFunctionType.Identity

            def proj_qk_chunk(p, which, qc, dve_evac=False):
                """Single (pair, chunk) Q/K projection through the pj pool.
                Evacuation on ACT by default; dve_evac=True moves it to the
                DVE for projections injected into ACT-floor units."""
                w, bias, dst, xt = (("wq", bq_sb, qT, xq_t) if which == "q"
                                    else ("wk", bk_sb, kT, xk_t))
                ps = pj.tile([128, 512], f32, tag="pj", name="ps")
                sl = cs(qc)
                for kt in range(KT):
                    nc.tensor.matmul(ps, w_sb[w][:, kt, p * 128:(p + 1) * 128],
                                     xt[kt][:, sl],
                                     start=(kt == 0), stop=(kt == KT - 1))
                if dve_evac:
                    nc.vector.tensor_scalar(out=dst[p][:, sl], in0=ps,
                                            scalar1=bias[:, p:p + 1],
                                            scalar2=None, op0=ADD)
                else:
                    nc.scalar.activation(dst[p][:, sl], ps, IDENT,
                                         bias=bias[:, p:p + 1])

            def proj_v_tt(tts):
                """V projection for token tiles tts, both pairs at once."""
                for tt in tts:
                    ps = pj.tile([128, 512], f32, tag="pj", name="ps")
                    sl = slice(tt * 128, (tt + 1) * 128)
                    for kt in range(KT):
                        nc.tensor.matmul(ps[:, 0:DLOC], xv_t[kt][:, sl],
                                         w_sb["wv"][:, kt, :],
                                         start=(kt == 0), stop=(kt == KT - 1))
                    for p in range(NP):
                        nc.vector.tensor_tensor(
                            out=vv[p][:, :, tt, 0:64],
                            in0=ps[:, p * 128:(p + 1) * 128],
                            in1=bv_bc[:, p * 128:(p + 1) * 128], op=ADD)

            def outproj_tt(tts):
                for tt in tts:
                    tsl = slice(tt * 128, (tt + 1) * 128)
                    for eh in range(2):
                        po = pj.tile([128, 512], f32, tag="pj", name="po")
                        esl = slice(eh * 512, (eh + 1) * 512)
                        for p in range(NP):
                            nc.tensor.matmul(po, ctxn[p][:, tsl], wo_sb[p][:, esl],
                                             start=(p == 0), stop=(p == NP - 1))
                        oc = osb.tile([128, 512], f16, tag="oc", name="oc")
                        nc.vector.tensor_copy(oc, po)
                        nc.sync.dma_start(out=out[tsl, esl], in_=oc)

            # ---- attention ----------------------------------------------
            # kb-granular pipeline: one psco tile [128, h0-q|h1-q] per key
            # block, ring of 2 => scores(kb) only waits on exp(kb-2), which
            # the ACT engine finished long ago. One exp ACTIVATE per kb
            # covers both heads (and the mask bias, when present).
            def attn_qc(p, qc, injects=(), last=False, pj_head=False):
                injects = list(injects)
                qsl = cs(qc)
                pctx = [pcx.tile([65, 512], f32, tag=f"cx{h}", name=f"cx{h}")
                        for h in range(2)]

                def scores_kb(kb, use_pj=False):
                    ksl = slice(kb * 128, (kb + 1) * 128)
                    if use_pj:
                        # kb0/kb1 through the pj pool: the unit's start no
                        # longer waits on the previous unit's exp(14)/(15)
                        # freeing the psc ring
                        ts = [pj.tile([128, 512], f32, tag="pj", name="scp")
                              for _ in range(2)]
                        for h in range(2):
                            hsl = slice(h * 64, (h + 1) * 64)
                            nc.tensor.matmul(
                                ts[h], kT[p][hsl, ksl], qT[p][hsl, qsl],
                                start=True, stop=True,
                                tile_position=(h * 64, 0))
                        return tuple(ts)
                    psco = psc.tile([128, 1024], f32, tag="sc", name="sc")
                    for h in range(2):
                        hsl = slice(h * 64, (h + 1) * 64)
                        nc.tensor.matmul(
                            psco[:, h * 512:(h + 1) * 512],
                            kT[p][hsl, ksl], qT[p][hsl, qsl],
                            start=True, stop=True,
                            tile_position=(h * 64, 0))
                    return psco

                def exp_ctx_kb(kb, psco):
                    et = esb.tile([128, 1024], f16, tag="e", name="et")
                    if isinstance(psco, tuple):
                        for h in range(2):
                            esl = slice(h * 512, (h + 1) * 512)
                            if masked:
                                nc.scalar.activation(et[:, esl], psco[h], EXP,
                                                     bias=mb_sb[:, kb:kb + 1],
                                                     scale=SCALE)
                            else:
                                nc.scalar.activation(et[:, esl], psco[h], EXP,
                                                     scale=SCALE)
                    elif masked:
                        nc.scalar.activation(et, psco, EXP,
                                             bias=mb_sb[:, kb:kb + 1], scale=SCALE)
                    else:
                        nc.scalar.activation(et, psco, EXP, scale=SCALE)
                    for h in range(2):
                        nc.tensor.matmul(
                            pctx[h], vv[p][:, h, kb, 0:65],
                            et[:, h * 512:(h + 1) * 512],
                            start=(kb == 0), stop=(kb == NKB - 1))

                ring = [scores_kb(0, use_pj=pj_head),
                        scores_kb(1, use_pj=pj_head)]
                for kb in range(2, NKB):
                    ring.append(scores_kb(kb))
                    if injects:
                        f = injects.pop(0)
                        if f is not None:
                            f()
                    exp_ctx_kb(kb - 2, ring.pop(0))
                while injects:
                    f = injects.pop(0)
                    if f is not None:
                        f()
                exp_ctx_kb(NKB - 2, ring.pop(0))
                exp_ctx_kb(NKB - 1, ring.pop(0))

                # evacuate pctx first (both heads) so the next unit's ctx
                # chain gets its PSUM banks back ~2.5us earlier; the
                # recip/broadcast/multiply then run off the SBUF copies.
                # In the LAST unit nothing reuses the banks, so skip the
                # cd copies (multiply straight from PSUM) and put the cl
                # copies on the now-idle scalar engine.
                cds, cls = [], []
                for h in range(2):
                    if last:
                        # nothing reuses the banks: multiply straight from
                        # PSUM; only the denominator row is copied out (the
                        # DVE reciprocal can't read PSUM)
                        cds.append(pctx[h][0:64, :])
                        cl = nrm.tile([1, 512], f32, tag=f"cl{h}", name="cl")
                        nc.scalar.copy(cl, pctx[h][64:65, :])
                        cls.append(cl)
                    else:
                        cd = nrm.tile([64, 512], f32, tag=f"cd{h}", name="cd")
                        nc.vector.tensor_copy(cd, pctx[h][0:64, :])
                        cds.append(cd)
                        cl = nrm.tile([1, 512], f32, tag=f"cl{h}", name="cl")
                        nc.vector.tensor_copy(cl, pctx[h][64:65, :])
                        cls.append(cl)
                for h in range(2):
                    hsl = slice(h * 64, (h + 1) * 64)
                    rl = nrm.tile([1, 512], f32, tag="rl", name="rl")
                    nc.vector.reciprocal_approx_fast(rl, cls[h])
                    rl_bc = nrm.tile([64, 512], f32, tag="rlb", name="rlb")
                    nc.gpsimd.partition_broadcast(rl_bc, rl)
                    nc.vector.tensor_tensor(out=ctxn[p][hsl, qsl],
                                            in0=cds[h], in1=rl_bc, op=MUL)

            # ---- prologue: immediate PE p-state warmup on the junk tile,
            # then chunk-0 K/Q projections as soon as their DMAs land.
            # enough junk matmuls to hold the PE p-state at full clock until
            # xk chunk 0 lands (~21us); each costs ~215ns hot, and at most
            # one junk matmul of latency is added once real work is ready.
            ksc = psc.tile([128, 1024], f32, tag="sc", name="ksc")
            for wu in range(42):
                nc.tensor.matmul(ksc[:, 0:512], junk[:, 0:128],
                                 junk[:, 128:640],
                                 start=True, stop=True)
            proj_qk_chunk(0, "k", 0)
            proj_qk_chunk(0, "q", 0)
            # late-issued xv_c2/c3 on scalar: emitted after the Q(0,0)
            # evac so the scheduler is less likely to hoist them in front
            # of the ACT evacuations
            nc.scalar.dma_start(out=xv_b[:, :, cs(2)], in_=xvr[:, :, cs(2)])
            nc.scalar.dma_start(out=xv_b[:, :, cs(3)], in_=xvr[:, :, cs(3)])

            # ---- attention schedule with injected projection work -------
            # unit 0 carries the V projection (deadline: exp_ctx(kb) needs
            # vv[kb]) plus just-in-time K chunks (scores(kb) needs chunk
            # kb//4) and Q(0,1); later units are balanced ~6.6us each.
            # emission-order deadlines inside unit 0: inject[i] is emitted
            # after scores(i+2), so K(c1) needs i<=1 (scores(4) reads kT
            # chunk 1), K(c2) i<=5, K(c3) i<=9, V(t,t+1) i<=t (exp_ctx(t)
            # reads vv[t]).
            attn_qc(0, 0, [
                lambda: proj_v_tt([0, 1]),
                lambda: proj_qk_chunk(0, "k", 1),
                lambda: proj_v_tt([2, 3]),
                lambda: proj_v_tt([4, 5]),
                lambda: proj_qk_chunk(0, "k", 2),
                lambda: proj_v_tt([6, 7]),
                lambda: proj_v_tt([8, 9]),
                lambda: proj_v_tt([10, 11]),
                lambda: proj_qk_chunk(0, "k", 3),
                lambda: proj_v_tt([12, 13]),
                lambda: proj_v_tt([14, 15]),
                lambda: proj_qk_chunk(0, "q", 1),
                # pair-1 K projections ride unit0's DMA-stall bubbles
                lambda: proj_qk_chunk(1, "k", 0),
                lambda: proj_qk_chunk(1, "k", 1),
            ])
            attn_qc(0, 1, [
                lambda: proj_qk_chunk(0, "q", 2),
                lambda: proj_qk_chunk(0, "q", 3),
            ])
            attn_qc(0, 2, [
                lambda: proj_qk_chunk(1, "k", 2),
                lambda: proj_qk_chunk(1, "k", 3),
            ])
            attn_qc(0, 3, [
                lambda: proj_qk_chunk(1, "q", 0),
                lambda: proj_qk_chunk(1, "q", 1),
            ])
            attn_qc(1, 0, [
                lambda: proj_qk_chunk(1, "q", 2),
                lambda: proj_qk_chunk(1, "q", 3),
            ])
            # outproj injected a few slots in so it never stalls the PE on
            # the previous unit's normalize chain (ctxn written ~3us after
            # that unit's last exp)
            attn_qc(1, 1, [
                None,
                None,
                lambda: outproj_tt([0]),
                None,
                lambda: outproj_tt([1]),
                None,
                lambda: outproj_tt([2]),
                None,
                lambda: outproj_tt([3]),
            ])
            attn_qc(1, 2, [
                None,
                None,
                lambda: outproj_tt([4]),
                None,
                lambda: outproj_tt([5]),
                None,
                lambda: outproj_tt([6]),
                None,
                lambda: outproj_tt([7]),
            ])
            attn_qc(1, 3, [
                None,
                None,
                lambda: outproj_tt([8]),
                None,
                lambda: outproj_tt([9]),
                None,
                lambda: outproj_tt([10]),
                None,
                lambda: outproj_tt([11]),
            ], last=True)

            def outproj_tail(tts, store_engs):
                for tt in tts:
                    tsl = slice(tt * 128, (tt + 1) * 128)
                    po = psc.tile([128, 1024], f32, tag="sc", name="po2")
                    for eh in range(2):
                        esl = slice(eh * 512, (eh + 1) * 512)
                        for p in range(NP):
                            nc.tensor.matmul(po[:, esl], ctxn[p][:, tsl],
                                             wo_sb[p][:, esl],
                                             start=(p == 0), stop=(p == NP - 1))
                    for eh in range(2):
                        esl = slice(eh * 512, (eh + 1) * 512)
                        oc = osb.tile([128, 512], f16, tag="oc", name="oc")
                        nc.vector.tensor_copy(oc, po[:, esl])
                        store_engs[eh].dma_start(out=out[tsl, esl], in_=oc)

            outproj_tail([12, 13], store_engs=[nc.scalar, nc.gpsimd])
            outproj_tail([14, 15], store_engs=[nc.sync, nc.gpsimd])

    nc.compile()
    return nc


def _get_prog(masked: bool):
    key = masked
    if key not in _prog_cache:
        _prog_cache[key] = _build(masked)
    return _prog_cache[key]


def make_in_maps(query, key, value, mask, Wq, bq, Wk, bk, Wv, bv, Wo, bo):
    query = np.asarray(query)
    key = np.asarray(key)
    value = np.asarray(value)
    mask = np.asarray(mask)
    Wq, bq = np.asarray(Wq), np.asarray(bq)
    Wk, bk = np.asarray(Wk), np.asarray(bk)
    Wv, bv = np.asarray(Wv), np.asarray(bv)
    Wo = np.asarray(Wo)

    def t16(x):  # [S, B, D] -> contiguous [D, B, S] fp16
        return np.ascontiguousarray(x.transpose(2, 1, 0).astype(np.float16))

    def warr(W, hs):  # [128, KT*DLOC]: row p = concat_kt W[hs+m, kt*128+p]
        wt = W[hs:hs + DLOC, :].T.astype(np.float16)       # [kt*128+p, m]
        return np.ascontiguousarray(
            wt.reshape(KT, 128, DLOC).transpose(1, 0, 2).reshape(128, KT * DLOC))

    xq3, xk3, xv3 = t16(query), t16(key), t16(value)
    xqb = [np.ascontiguousarray(xq3[:, b, :]) for b in range(B)]
    xkb = [np.ascontiguousarray(xk3[:, b, :]) for b in range(B)]
    xvb = [np.ascontiguousarray(xv3[:, b, :]) for b in range(B)]
    mbias = np.where(mask.reshape(S), 0.0, -1e30).astype(np.float32)

    wqs = [warr(Wq, g * DLOC) for g in range(4)]
    wks = [warr(Wk, g * DLOC) for g in range(4)]
    wvs = [warr(Wv, g * DLOC) for g in range(4)]
    wos = [np.ascontiguousarray(Wo[:, g * DLOC:(g + 1) * DLOC].T.astype(np.float16))
           for g in range(4)]

    in_maps = []
    for c in range(NCORES):
        b, g = c // 4, c % 4
        hs = g * DLOC
        in_maps.append({
            "xq": xqb[b], "xk": xkb[b], "xv": xvb[b],
            "wq": wqs[g], "wk": wks[g], "wv": wvs[g], "wo": wos[g],
            "bq": bq[hs:hs + DLOC].astype(np.float32),
            "bk": bk[hs:hs + DLOC].astype(np.float32),
            "bv": bv[hs:hs + DLOC].astype(np.float32),
            "mb": mbias,
        })
    return in_maps


def kernel(query, key, value, mask, Wq, bq, Wk, bk, Wv, bv, Wo, bo):
    from concourse.bass_utils import run_bass_kernel_spmd

    mask = np.asarray(mask)
    bo = np.asarray(bo)
    masked = not bool(mask.all())
    nc = _get_prog(masked)
    in_maps = make_in_maps(query, key, value, mask, Wq, bq, Wk, bk, Wv, bv, Wo, bo)

    res = run_bass_kernel_spmd(nc, in_maps, core_ids=list(range(NCORES)))
    acc = np.zeros((S, B, D), dtype=np.float64)
    for c in range(NCORES):
        acc[:, c // 4, :] += res.results[c]["out"].astype(np.float64)
    acc += bo.astype(np.float64)
    return acc.astype(np.float32)
